# revision 33
# baseline (speedup 1.0000x reference)
"""Sparse window attention (NMS-selected windows) for Trainium2, 8 cores.

Strategy:
- Host: replicate the (tiny) score/NMS control flow bit-exactly with jax-CPU,
  build per-batch gather/scatter token tables and 1/count patches; quantize x
  to fp8 (x*S_P); add x to the device result at the end (the device computes
  only the window contributions, scattered onto a zero-initialized output).
- Device (1 batch per NeuronCore, SPMD over 8 cores), software-pipelined over
  windows: iteration i runs front(i) [gather -> fp8 ROI -> fp8 qkv], mid(i-1)
  [scores -> exp], back(i-2) [attnV -> normalize -> DMA-transpose -> out proj
  -> scatter-accumulate], so PE/Act/DVE/Pool all stay busy on different
  windows. PSUM is tiled in 1-bank [128,512] units split across per-stage
  pools; evacuations are balanced across DVE / Act / Pool(gpsimd).
"""

import os
import sys

sys.path.insert(0, "/opt/trn_rl_repo")

_ABLATE = os.environ.get("KERNEL_ABLATE", "")

import numpy as np

import concourse.bass as bass
import concourse.bacc as bacc
import concourse.mybir as mybir
import concourse.tile as tile
from concourse.bass_utils import run_bass_kernel_spmd
from concourse.masks import make_identity

f32 = mybir.dt.float32
i32 = mybir.dt.int32

# ---- problem constants (must match reference.py) ----
H = W = 128
WIN = 16
DIM = 512
HEADS = 8
DIM_HEAD = 64
INNER = HEADS * DIM_HEAD
SCALE = DIM_HEAD ** -0.5
KEEP = 44
IOU_T = 0.2
NB = 8
NTOK = H * W  # 16384
M = 225

# static shifted windows
_s = np.arange(0, H - WIN + 1, WIN // 2)
_sx, _sy = np.meshgrid(_s, _s)
WINDOWS = np.stack(
    [_sx.ravel(), _sy.ravel(), _sx.ravel() + WIN - 1, _sy.ravel() + WIN - 1], 1
).astype(np.float32)
SX_NP = WINDOWS[:, 0].astype(np.int32)
SY_NP = WINDOWS[:, 1].astype(np.int32)

_x1, _y1, _x2, _y2 = WINDOWS[:, 0], WINDOWS[:, 1], WINDOWS[:, 2], WINDOWS[:, 3]
_area = (_x2 - _x1) * (_y2 - _y1)
_iw = np.clip(np.minimum(_x2[:, None], _x2[None]) - np.maximum(_x1[:, None], _x1[None]), 0, None)
_ih = np.clip(np.minimum(_y2[:, None], _y2[None]) - np.maximum(_y1[:, None], _y1[None]), 0, None)
_inter = _iw * _ih
IOU_NP = (_inter / (_area[:, None] + _area[None] - _inter)).astype(np.float32)

_bin = (WIN - 1.0) / WIN
_r = (np.arange(WIN) + 0.5) * _bin
_q = np.floor(_r).astype(int)
_f = (_r - _q).astype(np.float32)
_A = np.zeros((WIN, WIN), np.float32)
_A[np.arange(WIN), _q] = 1.0 - _f
_A[np.arange(WIN), np.minimum(_q + 1, WIN - 1)] += _f
BILIN_NP = _A  # (16,16)


def _nms_select_numpy(prob, w_fix):
    """Numpy fallback replica of the reference score+NMS."""
    entropy = -np.sum(prob * np.log2(prob + np.float32(1e-10)), axis=1)
    k = w_fix[0, 0]
    sc = np.zeros((NB, 15, 15), np.float32)
    for i in range(15):
        for j in range(15):
            sc[:, i, j] = np.sum(
                entropy[:, i * 4:i * 4 + 8, j * 4:j * 4 + 8] * k[None], axis=(1, 2)
            )
    score = (sc / np.float32(64.0)).reshape(NB, -1)
    out = np.zeros((NB, KEEP), np.int64)
    for b in range(NB):
        order = np.argsort(-score[b], kind="stable")
        iou_s = IOU_NP[order][:, order]
        supp = np.zeros(M, bool)
        for i in range(M):
            if not supp[i]:
                supp |= (iou_s[i] > IOU_T) & (np.arange(M) > i)
        pos = np.where(~supp, np.arange(M), M)
        pos = np.minimum(np.sort(pos)[:KEEP], M - 1)
        out[b] = order[pos]
    return out


def _nms_select_host(prob, w_fix):
    """Bit-exact replica of reference score+NMS, on jax CPU. Returns (NB, KEEP) int."""
    try:
        import jax
    except ImportError:
        return _nms_select_numpy(prob, w_fix)
    import jax.numpy as jnp
    from jax import lax

    cpu = jax.devices("cpu")[0]
    with jax.default_device(cpu):
        probj = jnp.asarray(prob)
        entropy = -jnp.sum(probj * jnp.log2(probj + 1e-10), axis=1)
        score = lax.conv(entropy[:, None], jnp.asarray(w_fix), (WIN // 4, WIN // 4), "VALID")
        score = (score / float((WIN // 2) ** 2)).reshape(NB, -1)
        iou = jnp.asarray(IOU_NP)

        def one(sc):
            order = jnp.argsort(-sc)
            iou_s = iou[order][:, order]

            def body(i, supp):
                return supp | ((~supp[i]) & (iou_s[i] > IOU_T) & (jnp.arange(M) > i))

            supp = lax.fori_loop(0, M, body, jnp.zeros((M,), dtype=bool))
            pos = jnp.where(~supp, jnp.arange(M), M)
            pos = jnp.minimum(jnp.sort(pos)[:KEEP], M - 1)
            return order[pos]

        idx = jax.vmap(one)(score)
        return np.asarray(jax.device_get(idx))


# fp8 scale factors (powers of two; results rescaled exactly)
S_P = 16.0   # x8 / patT (roi-resampled activations)
S_T = 64.0   # bilinear resample table
S_W = 64.0   # wqkv / wout weights
S_QK = 8.0   # q, k
S_V = 16.0   # v
S_O = 32.0   # attention output (outT); ones col = S_V/S_O so out_q lands at S_O

VST = 66     # per-head stride in the v tile: 64 dims + ones col + pad


def _build_program(repeat=1):
    """Build+compile the shared SPMD Bass program (one batch per core).

    repeat>1 wraps the whole computation in an on-device loop (timing use
    only: out accumulates repeat times so results are wrong, but the
    per-iteration work is identical)."""
    nc = bacc.Bacc(
        "TRN2", target_bir_lowering=False, debug=False, num_devices=NB,
    )

    bf16 = mybir.dt.bfloat16
    f8 = mybir.dt.float8e4

    x8 = nc.dram_tensor("x8", [NTOK, DIM], f8, kind="ExternalInput")
    wqkv8d = nc.dram_tensor("wqkv8", [128, 4 * 3 * INNER], f8, kind="ExternalInput")
    wout8d = nc.dram_tensor("woutb", [128, 4 * DIM], bf16, kind="ExternalInput")
    tkT8d = nc.dram_tensor("tkT8", [128, 512], f8, kind="ExternalInput")
    gsidx = nc.dram_tensor("gsidx", [128, 2 * KEEP], i32, kind="ExternalInput")
    invp = nc.dram_tensor("invp", [128, 2 * KEEP], f32, kind="ExternalInput")
    out = nc.dram_tensor("out", [NTOK, DIM], f32, kind="ExternalOutput")
    outw = nc.dram_tensor("outw", [256, DIM], f32) if _ABLATE else None

    EXP = mybir.ActivationFunctionType.Exp
    CPY = mybir.ActivationFunctionType.Copy
    MUL = mybir.AluOpType.mult
    DR = mybir.MatmulPerfMode.DoubleRow

    def kp(tile2d, offset, kstride, n):
        """AP [128p, 2(ktile)@kstride, n@1] for DoubleRow k-tile pairs."""
        base = tile2d[:, offset: offset + 1]
        return bass.AP(
            tensor=base.tensor, offset=base.offset,
            ap=[base.ap[0], [kstride, 2], [1, n]],
        )

    def strided(tile2d, offset, dims):
        base = tile2d[:, offset: offset + 1]
        return bass.AP(
            tensor=base.tensor, offset=base.offset,
            ap=[base.ap[0]] + [[s, n] for s, n in dims],
        )

    with tile.TileContext(nc) as tc:
        with (
            tc.tile_pool(name="cst", bufs=1) as cst,
            tc.tile_pool(name="sb", bufs=2) as sb,
            tc.tile_pool(name="pf", bufs=2, space="PSUM") as pf,
            tc.tile_pool(
                name="pm", bufs=(1 if "petrans" in _ABLATE else 2), space="PSUM"
            ) as pm,
            tc.tile_pool(name="pb", bufs=2, space="PSUM") as pb,
        ):
            wqkv_sb = cst.tile([128, 4 * 3 * INNER], f8)
            nc.sync.dma_start(wqkv_sb[:], wqkv8d[:])
            wout_sb = cst.tile([128, 4 * DIM], bf16)
            nc.sync.dma_start(wout_sb[:], wout8d[:])
            tkT_sb = cst.tile([128, 512], f8)
            nc.sync.dma_start(tkT_sb[:], tkT8d[:])
            idx_sb = cst.tile([128, 2 * KEEP], i32)
            nc.sync.dma_start(idx_sb[:], gsidx[:])
            inv_sb = cst.tile([128, 2 * KEEP], f32)
            nc.sync.dma_start(inv_sb[:], invp[:])
            # denominator ones-column at S_V/S_O so out_q = attn_out * S_O
            ones_c = cst.tile([128, 16], f8)
            nc.vector.memset(ones_c[:], S_V / S_O)
            if "petrans" in _ABLATE:
                ident = cst.tile([128, 128], bf16)
                make_identity(nc, ident[:])

            qsc = S_QK / (S_P * S_W)

            def issue_gather(w):
                # token-major gather: partition p = token c*128+p of window w,
                # both halves fused into one SWDGE instruction
                patch = sb.tile([128, 1024], f8, tag="patch", bufs=5,
                                name=f"patch_{w}")
                if _ABLATE == "directgather":
                    for c in range(2):
                        nc.gpsimd.dma_start(
                            patch[:, c * 512:(c + 1) * 512],
                            x8[(2 * w + c) * 128:(2 * w + c + 1) * 128, :],
                        )
                elif "fusedgather" not in _ABLATE:
                    for c in range(2):
                        nc.gpsimd.indirect_dma_start(
                            out=patch[:, c * 512:(c + 1) * 512],
                            out_offset=None,
                            in_=x8[:, :],
                            in_offset=bass.IndirectOffsetOnAxis(
                                ap=idx_sb[:, 2 * w + c: 2 * w + c + 1], axis=0
                            ),
                        )
                else:
                    nc.gpsimd.indirect_dma_start(
                        out=strided(patch, 0, [[512, 2], [1, 512]]),
                        out_offset=None,
                        in_=x8[:, :],
                        in_offset=bass.IndirectOffsetOnAxis(
                            ap=idx_sb[:, 2 * w: 2 * w + 2], axis=0
                        ),
                    )
                return patch

            PF = 3  # gather prefetch depth

            st = {}

            def stage_front(w, patch):
                """fp8 ROI (DR) -> qkv projections."""
                patT8 = sb.tile([128, 1024], f8, tag="patT", bufs=3)
                for t in range(2):
                    psA = pf.tile([128, 512], f32, tag="pf")
                    for j in range(2):
                        dm = 2 * t + j
                        nc.tensor.matmul(
                            psA[:, j * 256:(j + 1) * 256],
                            lhsT=kp(patch, dm * 128, 512, 128),
                            rhs=kp(tkT_sb, 0, 256, 256),
                            start=True, stop=True, perf_mode=DR,
                        )
                    nc.vector.tensor_scalar(
                        patT8[:, t * 512:(t + 1) * 512], psA[:, :],
                        1.0 / S_T, None, MUL,
                    )

                # ---- q,k (fp8 DoubleRow): qkT8[inner, tok] = x S_QK
                qkT8 = sb.tile([128, 2048], f8, tag="qkT", bufs=3)
                for t in range(4):
                    psB = pf.tile([128, 512], f32, tag="pf")
                    for j in range(2):
                        mi = 2 * t + j
                        for dkp in range(2):
                            nc.tensor.matmul(
                                psB[:, j * 256:(j + 1) * 256],
                                lhsT=kp(wqkv_sb, dkp * 2 * 1536 + mi * 128, 1536, 128),
                                rhs=kp(patT8, dkp * 2 * 256, 256, 256),
                                start=(dkp == 0),
                                stop=(dkp == 1),
                                perf_mode=DR,
                            )
                    dst = qkT8[:, t * 512:(t + 1) * 512]
                    if t < 2:
                        nc.vector.tensor_scalar(dst, psB[:, :], qsc, None, MUL)
                    else:
                        nc.scalar.activation(dst, psB[:, :], CPY, scale=qsc)

                # ---- v (fp8 DoubleRow) in [token, inner] layout, x S_V
                v_sb = sb.tile([128, 2 * 8 * VST], f8, tag="v", bufs=4)
                for t in range(2):
                    psV = pf.tile([128, 512], f32, tag="pf")
                    for dkp in range(2):
                        nc.tensor.matmul(
                            psV[:, :],
                            lhsT=kp(patT8, dkp * 2 * 256 + t * 128, 256, 128),
                            rhs=kp(wqkv_sb, dkp * 2 * 1536 + 1024, 1536, 512),
                            start=(dkp == 0),
                            stop=(dkp == 1),
                            perf_mode=DR,
                        )
                    nc.vector.tensor_scalar(
                        strided(v_sb, t * 8 * VST, [[VST, 8], [1, 64]]),
                        psV[:, :], S_V / (S_P * S_W), None, MUL,
                    )
                nc.gpsimd.tensor_copy(
                    strided(v_sb, 64, [[8 * VST, 2], [VST, 8], [1, 2]]),
                    strided(ones_c, 0, [[0, 2], [1, 16]]),
                )
                st[w] = {"qkT8": qkT8, "v_sb": v_sb}

            def stage_mid(w, half):
                """scores + exp for head-pairs 2*half, 2*half+1 (Act-heavy)."""
                qkT8 = st[w]["qkT8"]
                if half == 0:
                    st[w]["e_all"] = sb.tile(
                        [128, 8 * 512], f8, tag="e", bufs=3, name=f"e_{w}"
                    )
                e_all = st[w]["e_all"]
                for j in range(half * 2, half * 2 + 2):
                    psS = pm.tile([128, 1024], f32, tag="pm")
                    for hh in range(2):
                        h = 2 * j + hh
                        po = (h % 2) * 64
                        qb = h // 2
                        kb = 4 + h // 2
                        for kt in range(2):
                            nc.tensor.matmul(
                                psS[:, hh * 512 + kt * 256: hh * 512 + (kt + 1) * 256],
                                lhsT=qkT8[po:po + 64, kb * 256 + kt * 128: kb * 256 + (kt + 1) * 128],
                                rhs=qkT8[po:po + 64, qb * 256:(qb + 1) * 256],
                                start=True,
                                stop=True,
                            )
                    nc.scalar.activation(
                        e_all[:, 2 * j * 512:(2 * j + 2) * 512], psS[:, :], EXP,
                        scale=SCALE / (S_QK * S_QK),
                    )

            def stage_back(w):
                """attnV -> normalize -> DMA-transpose -> proj -> scatter."""
                e_all = st[w]["e_all"]
                v_sb = st[w]["v_sb"]
                outT = sb.tile([128, 1024], bf16, tag="outT")
                for m in range(2):
                    out_q = sb.tile([128, 512], bf16, tag=f"outq{m}",
                                    name=f"outq{m}_{w}")
                    for hg in range(2):
                        psAV = pb.tile([128, 512], f32, tag="av")
                        for h4 in range(4):
                            h = hg * 4 + h4
                            nc.tensor.matmul(
                                psAV[:, h4 * VST: h4 * VST + 66],
                                lhsT=kp(e_all, h * 512 + m * 128, 256, 128),
                                rhs=kp(v_sb, h * VST, 8 * VST, 66),
                                start=True,
                                stop=True,
                                perf_mode=DR,
                            )
                        # normalize + 1/count fold:
                        #   out_q = psAV * (invp/denom) = attn_out*S_O*invp
                        # (invp is per-token, like 1/denom, so it commutes
                        # with the linear output projection; psF then needs
                        # only a plain copy to SBUF.)
                        rc = sb.tile([128, 4], f32, tag="rc")
                        nc.vector.reciprocal(
                            rc[:, 0:4], strided(psAV, 64, [[VST, 4]]),
                        )
                        rc2 = sb.tile([128, 4], f32, tag="rc2")
                        nc.gpsimd.tensor_tensor(
                            rc2[:, 0:4],
                            rc[:, 0:4],
                            strided(inv_sb, 2 * w + m, [[0, 4]]),
                            op=MUL,
                        )
                        nc.vector.tensor_tensor(
                            strided(out_q, hg * 256, [[64, 4], [1, 64]]),
                            strided(psAV, 0, [[VST, 4], [1, 64]]),
                            strided(rc2, 0, [[1, 4], [0, 64]]),
                            op=MUL,
                        )
                    # transpose on the DMA xbar: outT[p, ib*256+m*128+t]
                    #   = out_q[t, ib*128+p]   (bf16, HWDGE, no engine cost)
                    if "petrans" in _ABLATE:
                        if m == 0:
                            st[w]["psT"] = pb.tile(
                                [128, 1024], bf16, tag="pt", bufs=1,
                                name=f"psT_{w}",
                            )
                        psT = st[w]["psT"]
                        for ib in range(4):
                            nc.tensor.transpose(
                                psT[:, m * 512 + ib * 128: m * 512 + (ib + 1) * 128],
                                out_q[:, ib * 128:(ib + 1) * 128],
                                ident[:],
                            )
                    else:
                        nc.sync.dma_start(
                            strided(outT, m * 128, [[256, 4], [1, 128]]),
                            out_q[:, 0:512],
                            transpose=True,
                        )
                if "petrans" in _ABLATE:
                    nc.scalar.copy(
                        strided(outT, 0, [[128, 2], [256, 4], [1, 128]]),
                        st[w]["psT"][:, :],
                    )

                # ---- output projection (bf16); invp was folded into the
                # normalize above, so the evac is one plain Act copy
                final = sb.tile([128, 1024], f32, tag="final")
                psF = pm.tile([128, 1024], f32, tag="pm")
                for m in range(2):
                    for bk in range(4):
                        nc.tensor.matmul(
                            psF[:, m * 512:(m + 1) * 512],
                            lhsT=outT[:, bk * 256 + m * 128: bk * 256 + (m + 1) * 128],
                            rhs=wout_sb[:, bk * 512:(bk + 1) * 512],
                            start=(bk == 0),
                            stop=(bk == 3),
                        )
                nc.scalar.copy(final[:, :], psF[:, :])
                if _ABLATE == "nodma":
                    for c in range(2):
                        nc.sync.dma_start(
                            outw[c * 128:(c + 1) * 128, :],
                            final[:, c * 512:(c + 1) * 512],
                        )
                elif "fusedscatter" not in _ABLATE:
                    for c in range(2):
                        nc.gpsimd.indirect_dma_start(
                            out=out[:, :],
                            out_offset=bass.IndirectOffsetOnAxis(
                                ap=idx_sb[:, 2 * w + c: 2 * w + c + 1], axis=0
                            ),
                            in_=final[:, c * 512:(c + 1) * 512],
                            in_offset=None,
                            compute_op=mybir.AluOpType.add,
                        )
                else:
                    nc.gpsimd.indirect_dma_start(
                        out=out[:, :],
                        out_offset=bass.IndirectOffsetOnAxis(
                            ap=idx_sb[:, 2 * w: 2 * w + 2], axis=0
                        ),
                        in_=strided(final, 0, [[512, 2], [1, 512]]),
                        in_offset=None,
                        compute_op=mybir.AluOpType.add,
                    )
                del st[w]

            def body():
                # out starts zeroed (PJRT zero-donation); scatters accumulate
                # window contributions and the host adds x afterwards.
                # Software-pipelined: iteration i runs front(i), mid(i-1),
                # back(i-2) so each engine always has ready work.
                pf_map = {}
                for w in range(min(PF, KEEP)):
                    pf_map[w] = issue_gather(w)
                for i in range(KEEP + 2):
                    if 1 <= i < KEEP + 1:
                        stage_mid(i - 1, 0)
                    if i < KEEP:
                        patch = pf_map.pop(i)
                        if i + PF < KEEP:
                            pf_map[i + PF] = issue_gather(i + PF)
                        stage_front(i, patch)
                    if 1 <= i < KEEP + 1:
                        stage_mid(i - 1, 1)
                    if 2 <= i:
                        stage_back(i - 2)

            if repeat == 1:
                body()
            else:
                with tc.For_i(0, repeat, 1):
                    body()
    nc.compile()
    return nc


_NC_CACHE = {}


def _get_program(repeat=1):
    if repeat not in _NC_CACHE:
        _NC_CACHE[repeat] = _build_program(repeat)
    return _NC_CACHE[repeat]


def _host_aux(idx):
    """Per-batch gather/scatter token tables + inverse-count patches."""
    p = np.arange(256)
    sy = SY_NP[idx]  # (KEEP,)
    sx = SX_NP[idx]
    # token (w, t) for t = 0..255: global = (sy + t//16)*128 + sx + t%16
    tok = (sy[:, None] + p[None] // WIN) * W + sx[:, None] + p[None] % WIN  # (KEEP,256)
    cnt = np.zeros(NTOK, np.float32)
    np.add.at(cnt, tok.ravel(), 1.0)
    inv = (np.float32(1.0) / (cnt + np.float32(1e-10))).astype(np.float32)
    gs = np.zeros((128, 2 * KEEP), np.int32)
    iv = np.zeros((128, 2 * KEEP), np.float32)
    for c in range(2):
        gs[:, c::2] = tok[:, c * 128:(c + 1) * 128].T
        iv[:, c::2] = inv[tok[:, c * 128:(c + 1) * 128]].T
    return gs, iv


def prepare_in_maps(inputs):
    x = np.asarray(inputs["x"], dtype=np.float32)
    prob = np.asarray(inputs["prob"], dtype=np.float32)
    W_fix = np.asarray(inputs["W_fix"], dtype=np.float32)
    W_qkv = np.asarray(inputs["W_qkv"], dtype=np.float32)
    W_out = np.asarray(inputs["W_out"], dtype=np.float32)

    idx = _nms_select_host(prob, W_fix)  # (NB, KEEP)

    f8np = mybir.dt.np(mybir.dt.float8e4)
    # host-side SBUF layouts for the weights (pre-scaled, fp8-quantized)
    wqkv8 = np.ascontiguousarray(
        (W_qkv.T * S_W).reshape(4, 128, 3 * INNER).transpose(1, 0, 2)
        .reshape(128, 4 * 3 * INNER)
    ).astype(f8np)
    woutb = np.ascontiguousarray(
        (W_out.T * S_W).reshape(4, 128, DIM).transpose(1, 0, 2).reshape(128, 4 * DIM)
    ).astype(mybir.dt.np(mybir.dt.bfloat16))
    T = np.kron(BILIN_NP, BILIN_NP).astype(np.float32)  # (256 pq, 256 ij)
    TT = np.ascontiguousarray(T.T)  # (256 ij, 256 pq)
    tkT8 = np.ascontiguousarray(
        (TT * S_T).reshape(2, 128, 256).transpose(1, 0, 2).reshape(128, 512)
    ).astype(f8np)

    in_maps = []
    for b in range(NB):
        gs, iv = _host_aux(idx[b])
        in_maps.append(
            {
                "x8": np.ascontiguousarray(x[b] * S_P).astype(f8np),
                "wqkv8": wqkv8,
                "woutb": woutb,
                "tkT8": tkT8,
                "gsidx": gs,
                "invp": (iv / np.float32(S_O * S_W)).astype(np.float32),
            }
        )
    return in_maps


def kernel(x, prob, W_fix, W_qkv, W_out, b_out):
    x = np.asarray(x, dtype=np.float32)
    prob = np.asarray(prob, dtype=np.float32)
    W_fix = np.asarray(W_fix, dtype=np.float32)
    W_qkv = np.asarray(W_qkv, dtype=np.float32)
    W_out = np.asarray(W_out, dtype=np.float32)
    b_out = np.asarray(b_out, dtype=np.float32)

    idx = _nms_select_host(prob, W_fix)  # (NB, KEEP)

    nc = _get_program()
    in_maps = prepare_in_maps(
        {"x": x, "prob": prob, "W_fix": W_fix, "W_qkv": W_qkv, "W_out": W_out}
    )
    res = run_bass_kernel_spmd(nc, in_maps, core_ids=list(range(NB)))
    # device returns only the scattered window contributions (out starts
    # zeroed); x is added here
    out = x + np.stack(
        [res.results[b]["out"] for b in range(NB)], 0
    ).astype(np.float32)

    if np.any(b_out != 0.0):
        # bias contributes b_out once per covered token (cnt*inv == 1 exactly)
        for b in range(NB):
            gs, _ = _host_aux(idx[b])
            mask = np.zeros(NTOK, np.float32)
            mask[gs.ravel()] = 1.0
            out[b] += mask[:, None] * b_out[None, :]
    return out


# revision 51
# speedup vs baseline: 1.0486x; 1.0486x over previous
"""Sparse window attention (NMS-selected windows) for Trainium2, 8 cores.

Strategy:
- Host: replicate the (tiny) score/NMS control flow bit-exactly with jax-CPU,
  build per-batch gather/scatter token tables and 1/count patches; quantize x
  to fp8 (x*S_P); add x to the device result at the end (the device computes
  only the window contributions, scattered onto a zero-initialized output).
- Device (1 batch per NeuronCore, SPMD over 8 cores), software-pipelined over
  windows: iteration i runs front(i) [gather -> fp8 ROI -> fp8 qkv], mid(i-1)
  [scores -> exp], back(i-2) [attnV -> normalize -> DMA-transpose -> out proj
  -> scatter-accumulate], so PE/Act/DVE/Pool all stay busy on different
  windows. PSUM is tiled in 1-bank [128,512] units split across per-stage
  pools; evacuations are balanced across DVE / Act / Pool(gpsimd).
"""

import os
import sys

sys.path.insert(0, "/opt/trn_rl_repo")

_ABLATE = os.environ.get("KERNEL_ABLATE", "")

import numpy as np

import concourse.bass as bass
import concourse.bacc as bacc
import concourse.mybir as mybir
import concourse.tile as tile
from concourse.bass_utils import run_bass_kernel_spmd
from concourse.masks import make_identity

f32 = mybir.dt.float32
i32 = mybir.dt.int32

# ---- problem constants (must match reference.py) ----
H = W = 128
WIN = 16
DIM = 512
HEADS = 8
DIM_HEAD = 64
INNER = HEADS * DIM_HEAD
SCALE = DIM_HEAD ** -0.5
KEEP = 44
IOU_T = 0.2
NB = 8
NTOK = H * W  # 16384
M = 225

# static shifted windows
_s = np.arange(0, H - WIN + 1, WIN // 2)
_sx, _sy = np.meshgrid(_s, _s)
WINDOWS = np.stack(
    [_sx.ravel(), _sy.ravel(), _sx.ravel() + WIN - 1, _sy.ravel() + WIN - 1], 1
).astype(np.float32)
SX_NP = WINDOWS[:, 0].astype(np.int32)
SY_NP = WINDOWS[:, 1].astype(np.int32)

_x1, _y1, _x2, _y2 = WINDOWS[:, 0], WINDOWS[:, 1], WINDOWS[:, 2], WINDOWS[:, 3]
_area = (_x2 - _x1) * (_y2 - _y1)
_iw = np.clip(np.minimum(_x2[:, None], _x2[None]) - np.maximum(_x1[:, None], _x1[None]), 0, None)
_ih = np.clip(np.minimum(_y2[:, None], _y2[None]) - np.maximum(_y1[:, None], _y1[None]), 0, None)
_inter = _iw * _ih
IOU_NP = (_inter / (_area[:, None] + _area[None] - _inter)).astype(np.float32)

_bin = (WIN - 1.0) / WIN
_r = (np.arange(WIN) + 0.5) * _bin
_q = np.floor(_r).astype(int)
_f = (_r - _q).astype(np.float32)
_A = np.zeros((WIN, WIN), np.float32)
_A[np.arange(WIN), _q] = 1.0 - _f
_A[np.arange(WIN), np.minimum(_q + 1, WIN - 1)] += _f
BILIN_NP = _A  # (16,16)


def _nms_select_numpy(prob, w_fix):
    """Numpy fallback replica of the reference score+NMS."""
    entropy = -np.sum(prob * np.log2(prob + np.float32(1e-10)), axis=1)
    k = w_fix[0, 0]
    sc = np.zeros((NB, 15, 15), np.float32)
    for i in range(15):
        for j in range(15):
            sc[:, i, j] = np.sum(
                entropy[:, i * 4:i * 4 + 8, j * 4:j * 4 + 8] * k[None], axis=(1, 2)
            )
    score = (sc / np.float32(64.0)).reshape(NB, -1)
    out = np.zeros((NB, KEEP), np.int64)
    for b in range(NB):
        order = np.argsort(-score[b], kind="stable")
        iou_s = IOU_NP[order][:, order]
        supp = np.zeros(M, bool)
        for i in range(M):
            if not supp[i]:
                supp |= (iou_s[i] > IOU_T) & (np.arange(M) > i)
        pos = np.where(~supp, np.arange(M), M)
        pos = np.minimum(np.sort(pos)[:KEEP], M - 1)
        out[b] = order[pos]
    return out


def _nms_select_host(prob, w_fix):
    """Bit-exact replica of reference score+NMS, on jax CPU. Returns (NB, KEEP) int."""
    try:
        import jax
    except ImportError:
        return _nms_select_numpy(prob, w_fix)
    import jax.numpy as jnp
    from jax import lax

    cpu = jax.devices("cpu")[0]
    with jax.default_device(cpu):
        probj = jnp.asarray(prob)
        entropy = -jnp.sum(probj * jnp.log2(probj + 1e-10), axis=1)
        score = lax.conv(entropy[:, None], jnp.asarray(w_fix), (WIN // 4, WIN // 4), "VALID")
        score = (score / float((WIN // 2) ** 2)).reshape(NB, -1)
        iou = jnp.asarray(IOU_NP)

        def one(sc):
            order = jnp.argsort(-sc)
            iou_s = iou[order][:, order]

            def body(i, supp):
                return supp | ((~supp[i]) & (iou_s[i] > IOU_T) & (jnp.arange(M) > i))

            supp = lax.fori_loop(0, M, body, jnp.zeros((M,), dtype=bool))
            pos = jnp.where(~supp, jnp.arange(M), M)
            pos = jnp.minimum(jnp.sort(pos)[:KEEP], M - 1)
            return order[pos]

        idx = jax.vmap(one)(score)
        return np.asarray(jax.device_get(idx))


# fp8 scale factors (powers of two; results rescaled exactly)
S_P = 16.0   # x8 / patT (roi-resampled activations)
S_T = 64.0   # bilinear resample table
S_W = 64.0   # wqkv / wout weights
S_QK = 8.0   # q, k
S_V = 16.0   # v
S_O = 32.0   # attention output (outT); ones col = S_V/S_O so out_q lands at S_O

VST = 66     # per-head stride in the v tile: 64 dims + ones col + pad


def _build_program(repeat=1):
    """Build+compile the shared SPMD Bass program (one batch per core).

    repeat>1 wraps the whole computation in an on-device loop (timing use
    only: out accumulates repeat times so results are wrong, but the
    per-iteration work is identical)."""
    nc = bacc.Bacc(
        "TRN2", target_bir_lowering=False, debug=False, num_devices=NB,
    )

    bf16 = mybir.dt.bfloat16
    f8 = mybir.dt.float8e4

    x8 = nc.dram_tensor("x8", [NTOK, DIM], f8, kind="ExternalInput")
    wqkv8d = nc.dram_tensor("wqkv8", [128, 4 * 3 * INNER], f8, kind="ExternalInput")
    wout8d = nc.dram_tensor("woutb", [128, 4 * DIM], bf16, kind="ExternalInput")
    tkT8d = nc.dram_tensor("tkT8", [128, 512], f8, kind="ExternalInput")
    gsidx = nc.dram_tensor("gsidx", [128, 2 * KEEP], i32, kind="ExternalInput")
    goffd = nc.dram_tensor("goff", [1, 2 * KEEP], i32, kind="ExternalInput")
    soffd = nc.dram_tensor("soff", [1, 2 * KEEP], i32, kind="ExternalInput")
    invp = nc.dram_tensor("invp", [128, 2 * KEEP], f32, kind="ExternalInput")
    # 4 output accumulators: windows round-robin over them so the
    # per-tensor write-ordering chains interleave; host sums them
    outs = [
        nc.dram_tensor(f"out{s}", [NTOK, DIM], f32, kind="ExternalOutput")
        for s in range(4)
    ]
    outw = nc.dram_tensor("outw", [256, DIM], f32) if _ABLATE else None

    EXP = mybir.ActivationFunctionType.Exp
    CPY = mybir.ActivationFunctionType.Copy
    MUL = mybir.AluOpType.mult
    DR = mybir.MatmulPerfMode.DoubleRow

    def kp(tile2d, offset, kstride, n):
        """AP [128p, 2(ktile)@kstride, n@1] for DoubleRow k-tile pairs."""
        base = tile2d[:, offset: offset + 1]
        return bass.AP(
            tensor=base.tensor, offset=base.offset,
            ap=[base.ap[0], [kstride, 2], [1, n]],
        )

    def strided(tile2d, offset, dims):
        base = tile2d[:, offset: offset + 1]
        return bass.AP(
            tensor=base.tensor, offset=base.offset,
            ap=[base.ap[0]] + [[s, n] for s, n in dims],
        )

    with tile.TileContext(nc) as tc:
        with (
            tc.tile_pool(name="cst", bufs=1) as cst,
            tc.tile_pool(name="sb", bufs=2) as sb,
            tc.tile_pool(name="pf", bufs=2, space="PSUM") as pf,
            tc.tile_pool(
                name="pm", bufs=(1 if "petrans" in _ABLATE else 2), space="PSUM"
            ) as pm,
            tc.tile_pool(name="pb", bufs=2, space="PSUM") as pb,
        ):
            wqkv_sb = cst.tile([128, 4 * 3 * INNER], f8)
            nc.sync.dma_start(wqkv_sb[:], wqkv8d[:])
            wout_sb = cst.tile([128, 4 * DIM], bf16)
            nc.sync.dma_start(wout_sb[:], wout8d[:])
            tkT_sb = cst.tile([128, 512], f8)
            nc.sync.dma_start(tkT_sb[:], tkT8d[:])
            idx_sb = cst.tile([128, 2 * KEEP], i32)
            nc.sync.dma_start(idx_sb[:], gsidx[:])
            inv_sb = cst.tile([128, 2 * KEEP], f32)
            nc.sync.dma_start(inv_sb[:], invp[:])
            goff_sb = cst.tile([1, 2 * KEEP], i32)
            nc.sync.dma_start(goff_sb[:], goffd[:])
            # denominator ones-column at S_V/S_O so out_q = attn_out * S_O
            ones_c = cst.tile([128, 16], f8)
            nc.vector.memset(ones_c[:], S_V / S_O)
            if "petrans" in _ABLATE:
                ident = cst.tile([128, 128], bf16)
                make_identity(nc, ident[:])

            # window-row access pattern: 8 image rows x 16 tokens x 512 dims,
            # based at a register element-offset read from goff_sb
            rg_g = nc.sync.alloc_register("rg_gather")
            x8_t = x8[:, :].tensor
            WAP = [[W * DIM, 8], [DIM, 16], [1, DIM]]

            qsc = S_QK / (S_P * S_W)

            def issue_gather(w):
                # token-major gather: partition p = token c*128+p of window w,
                # both halves fused into one SWDGE instruction
                patch = sb.tile([128, 1024], f8, tag="patch", bufs=5,
                                name=f"patch_{w}")
                if _ABLATE == "directgather":
                    for c in range(2):
                        nc.gpsimd.dma_start(
                            patch[:, c * 512:(c + 1) * 512],
                            x8[(2 * w + c) * 128:(2 * w + c + 1) * 128, :],
                        )
                elif "dyngather" in _ABLATE:
                    # dynamic-offset HWDGE gather: the window is 16
                    # contiguous 16-token row segments; offset register is
                    # loaded from goff_sb (host-computed per window half);
                    # measured slower than SWDGE on HW — kept for ref
                    for c in range(2):
                        nc.sync.reg_load(
                            rg_g, goff_sb[0:1, 2 * w + c: 2 * w + c + 1]
                        )
                        nc.sync.dma_start(
                            patch[:, c * 512:(c + 1) * 512],
                            bass.AP(x8_t, rg_g, WAP),
                        )
                elif "fusedgather" not in _ABLATE:
                    for c in range(2):
                        nc.gpsimd.indirect_dma_start(
                            out=patch[:, c * 512:(c + 1) * 512],
                            out_offset=None,
                            in_=x8[:, :],
                            in_offset=bass.IndirectOffsetOnAxis(
                                ap=idx_sb[:, 2 * w + c: 2 * w + c + 1], axis=0
                            ),
                        )
                else:
                    nc.gpsimd.indirect_dma_start(
                        out=strided(patch, 0, [[512, 2], [1, 512]]),
                        out_offset=None,
                        in_=x8[:, :],
                        in_offset=bass.IndirectOffsetOnAxis(
                            ap=idx_sb[:, 2 * w: 2 * w + 2], axis=0
                        ),
                    )
                return patch

            PF = 3  # gather prefetch depth

            st = {}

            def stage_front(w, patch):
                """fp8 ROI (DR) -> qkv projections."""
                patT8 = sb.tile([128, 1024], f8, tag="patT", bufs=3)
                for t in range(2):
                    psA = pf.tile([128, 512], f32, tag="pf")
                    for j in range(2):
                        dm = 2 * t + j
                        nc.tensor.matmul(
                            psA[:, j * 256:(j + 1) * 256],
                            lhsT=kp(patch, dm * 128, 512, 128),
                            rhs=kp(tkT_sb, 0, 256, 256),
                            start=True, stop=True, perf_mode=DR,
                        )
                    nc.vector.tensor_scalar(
                        patT8[:, t * 512:(t + 1) * 512], psA[:, :],
                        1.0 / S_T, None, MUL,
                    )

                # ---- q,k (fp8 DoubleRow): qkT8[inner, tok] = x S_QK
                qkT8 = sb.tile([128, 2048], f8, tag="qkT", bufs=3)
                for t in range(4):
                    psB = pf.tile([128, 512], f32, tag="pf")
                    for j in range(2):
                        mi = 2 * t + j
                        for dkp in range(2):
                            nc.tensor.matmul(
                                psB[:, j * 256:(j + 1) * 256],
                                lhsT=kp(wqkv_sb, dkp * 2 * 1536 + mi * 128, 1536, 128),
                                rhs=kp(patT8, dkp * 2 * 256, 256, 256),
                                start=(dkp == 0),
                                stop=(dkp == 1),
                                perf_mode=DR,
                            )
                    dst = qkT8[:, t * 512:(t + 1) * 512]
                    if t < 2:
                        nc.vector.tensor_scalar(dst, psB[:, :], qsc, None, MUL)
                    else:
                        nc.scalar.activation(dst, psB[:, :], CPY, scale=qsc)

                # ---- v (fp8 DoubleRow) in [token, inner] layout, x S_V
                v_sb = sb.tile([128, 2 * 8 * VST], f8, tag="v", bufs=4)
                for t in range(2):
                    psV = pf.tile([128, 512], f32, tag="pf")
                    for dkp in range(2):
                        nc.tensor.matmul(
                            psV[:, :],
                            lhsT=kp(patT8, dkp * 2 * 256 + t * 128, 256, 128),
                            rhs=kp(wqkv_sb, dkp * 2 * 1536 + 1024, 1536, 512),
                            start=(dkp == 0),
                            stop=(dkp == 1),
                            perf_mode=DR,
                        )
                    nc.vector.tensor_scalar(
                        strided(v_sb, t * 8 * VST, [[VST, 8], [1, 64]]),
                        psV[:, :], S_V / (S_P * S_W), None, MUL,
                    )
                nc.gpsimd.tensor_copy(
                    strided(v_sb, 64, [[8 * VST, 2], [VST, 8], [1, 2]]),
                    strided(ones_c, 0, [[0, 2], [1, 16]]),
                )
                st[w] = {"qkT8": qkT8, "v_sb": v_sb}

            def stage_mid(w, half):
                """scores + exp for head-pairs 2*half, 2*half+1 (Act-heavy)."""
                qkT8 = st[w]["qkT8"]
                if half == 0:
                    st[w]["e_all"] = sb.tile(
                        [128, 8 * 512], f8, tag="e", bufs=3, name=f"e_{w}"
                    )
                e_all = st[w]["e_all"]
                for j in range(half * 2, half * 2 + 2):
                    psS = pm.tile([128, 1024], f32, tag="pm")
                    for hh in range(2):
                        h = 2 * j + hh
                        po = (h % 2) * 64
                        qb = h // 2
                        kb = 4 + h // 2
                        for kt in range(2):
                            nc.tensor.matmul(
                                psS[:, hh * 512 + kt * 256: hh * 512 + (kt + 1) * 256],
                                lhsT=qkT8[po:po + 64, kb * 256 + kt * 128: kb * 256 + (kt + 1) * 128],
                                rhs=qkT8[po:po + 64, qb * 256:(qb + 1) * 256],
                                start=True,
                                stop=True,
                            )
                    nc.scalar.activation(
                        e_all[:, 2 * j * 512:(2 * j + 2) * 512], psS[:, :], EXP,
                        scale=SCALE / (S_QK * S_QK),
                    )

            def stage_back(w):
                """attnV -> normalize -> DMA-transpose -> proj -> scatter."""
                e_all = st[w]["e_all"]
                v_sb = st[w]["v_sb"]
                outT = sb.tile([128, 1024], bf16, tag="outT")
                for m in range(2):
                    out_q = sb.tile([128, 512], bf16, tag=f"outq{m}",
                                    name=f"outq{m}_{w}")
                    for hg in range(2):
                        psAV = pb.tile([128, 512], f32, tag="av")
                        for h4 in range(4):
                            h = hg * 4 + h4
                            nc.tensor.matmul(
                                psAV[:, h4 * VST: h4 * VST + 66],
                                lhsT=kp(e_all, h * 512 + m * 128, 256, 128),
                                rhs=kp(v_sb, h * VST, 8 * VST, 66),
                                start=True,
                                stop=True,
                                perf_mode=DR,
                            )
                        # normalize + 1/count fold:
                        #   out_q = psAV * (invp/denom) = attn_out*S_O*invp
                        # (invp is per-token, like 1/denom, so it commutes
                        # with the linear output projection; psF then needs
                        # only a plain copy to SBUF.)
                        rc = sb.tile([128, 4], f32, tag="rc")
                        nc.vector.reciprocal(
                            rc[:, 0:4], strided(psAV, 64, [[VST, 4]]),
                        )
                        rc2 = sb.tile([128, 4], f32, tag="rc2")
                        nc.gpsimd.tensor_tensor(
                            rc2[:, 0:4],
                            rc[:, 0:4],
                            strided(inv_sb, 2 * w + m, [[0, 4]]),
                            op=MUL,
                        )
                        nc.vector.tensor_tensor(
                            strided(out_q, hg * 256, [[64, 4], [1, 64]]),
                            strided(psAV, 0, [[VST, 4], [1, 64]]),
                            strided(rc2, 0, [[1, 4], [0, 64]]),
                            op=MUL,
                        )
                    # transpose on the DMA xbar: outT[p, ib*256+m*128+t]
                    #   = out_q[t, ib*128+p]   (bf16, HWDGE, no engine cost)
                    if "petrans" in _ABLATE:
                        if m == 0:
                            st[w]["psT"] = pb.tile(
                                [128, 1024], bf16, tag="pt", bufs=1,
                                name=f"psT_{w}",
                            )
                        psT = st[w]["psT"]
                        for ib in range(4):
                            nc.tensor.transpose(
                                psT[:, m * 512 + ib * 128: m * 512 + (ib + 1) * 128],
                                out_q[:, ib * 128:(ib + 1) * 128],
                                ident[:],
                            )
                    else:
                        nc.sync.dma_start(
                            strided(outT, m * 128, [[256, 4], [1, 128]]),
                            out_q[:, 0:512],
                            transpose=True,
                        )
                if "petrans" in _ABLATE:
                    nc.scalar.copy(
                        strided(outT, 0, [[128, 2], [256, 4], [1, 128]]),
                        st[w]["psT"][:, :],
                    )

                # ---- output projection (bf16); invp was folded into the
                # normalize above, so the evac is one plain Act copy
                final = sb.tile([128, 1024], f32, tag="final")
                psF = pm.tile([128, 1024], f32, tag="pm")
                for m in range(2):
                    for bk in range(4):
                        nc.tensor.matmul(
                            psF[:, m * 512:(m + 1) * 512],
                            lhsT=outT[:, bk * 256 + m * 128: bk * 256 + (m + 1) * 128],
                            rhs=wout_sb[:, bk * 512:(bk + 1) * 512],
                            start=(bk == 0),
                            stop=(bk == 3),
                        )
                nc.scalar.copy(final[:, :], psF[:, :])
                if _ABLATE == "nodma":
                    for c in range(2):
                        nc.sync.dma_start(
                            outw[c * 128:(c + 1) * 128, :],
                            final[:, c * 512:(c + 1) * 512],
                        )
                else:
                    # indirect scatter-accumulate; round-robin over slabs so
                    # the per-tensor-region write-ordering chains interleave
                    accum = (
                        mybir.AluOpType.bypass
                        if "noaccum" in _ABLATE
                        else mybir.AluOpType.add
                    )
                    for c in range(2):
                        nc.gpsimd.indirect_dma_start(
                            out=outs[w % 4][:, :],
                            out_offset=bass.IndirectOffsetOnAxis(
                                ap=idx_sb[:, 2 * w + c: 2 * w + c + 1], axis=0
                            ),
                            in_=final[:, c * 512:(c + 1) * 512],
                            in_offset=None,
                            compute_op=accum,
                        )
                del st[w]

            def body():
                # out starts zeroed (PJRT zero-donation); scatters accumulate
                # window contributions and the host adds x afterwards.
                # Software-pipelined: iteration i runs front(i), mid(i-1),
                # back(i-2) so each engine always has ready work.
                pf_map = {}
                for w in range(min(PF, KEEP)):
                    pf_map[w] = issue_gather(w)
                for i in range(KEEP + 2):
                    if 1 <= i < KEEP + 1:
                        stage_mid(i - 1, 0)
                    if i < KEEP:
                        patch = pf_map.pop(i)
                        if i + PF < KEEP:
                            pf_map[i + PF] = issue_gather(i + PF)
                        stage_front(i, patch)
                    if 1 <= i < KEEP + 1:
                        stage_mid(i - 1, 1)
                    if 2 <= i:
                        stage_back(i - 2)

            if repeat == 1:
                body()
            else:
                with tc.For_i(0, repeat, 1):
                    body()
    nc.compile()
    return nc


_NC_CACHE = {}


def _get_program(repeat=1):
    if repeat not in _NC_CACHE:
        _NC_CACHE[repeat] = _build_program(repeat)
    return _NC_CACHE[repeat]


def _host_aux(idx):
    """Per-batch gather/scatter token tables + inverse-count patches."""
    p = np.arange(256)
    sy = SY_NP[idx]  # (KEEP,)
    sx = SX_NP[idx]
    # token (w, t) for t = 0..255: global = (sy + t//16)*128 + sx + t%16
    tok = (sy[:, None] + p[None] // WIN) * W + sx[:, None] + p[None] % WIN  # (KEEP,256)
    cnt = np.zeros(NTOK, np.float32)
    np.add.at(cnt, tok.ravel(), 1.0)
    inv = (np.float32(1.0) / (cnt + np.float32(1e-10))).astype(np.float32)
    gs = np.zeros((128, 2 * KEEP), np.int32)
    iv = np.zeros((128, 2 * KEEP), np.float32)
    for c in range(2):
        gs[:, c::2] = tok[:, c * 128:(c + 1) * 128].T
        iv[:, c::2] = inv[tok[:, c * 128:(c + 1) * 128]].T
    return gs, iv


def prepare_in_maps(inputs):
    x = np.asarray(inputs["x"], dtype=np.float32)
    prob = np.asarray(inputs["prob"], dtype=np.float32)
    W_fix = np.asarray(inputs["W_fix"], dtype=np.float32)
    W_qkv = np.asarray(inputs["W_qkv"], dtype=np.float32)
    W_out = np.asarray(inputs["W_out"], dtype=np.float32)

    idx = _nms_select_host(prob, W_fix)  # (NB, KEEP)

    f8np = mybir.dt.np(mybir.dt.float8e4)
    # host-side SBUF layouts for the weights (pre-scaled, fp8-quantized)
    wqkv8 = np.ascontiguousarray(
        (W_qkv.T * S_W).reshape(4, 128, 3 * INNER).transpose(1, 0, 2)
        .reshape(128, 4 * 3 * INNER)
    ).astype(f8np)
    woutb = np.ascontiguousarray(
        (W_out.T * S_W).reshape(4, 128, DIM).transpose(1, 0, 2).reshape(128, 4 * DIM)
    ).astype(mybir.dt.np(mybir.dt.bfloat16))
    T = np.kron(BILIN_NP, BILIN_NP).astype(np.float32)  # (256 pq, 256 ij)
    TT = np.ascontiguousarray(T.T)  # (256 ij, 256 pq)
    tkT8 = np.ascontiguousarray(
        (TT * S_T).reshape(2, 128, 256).transpose(1, 0, 2).reshape(128, 512)
    ).astype(f8np)

    in_maps = []
    for b in range(NB):
        gs, iv = _host_aux(idx[b])
        sy = SY_NP[idx[b]]
        sx = SX_NP[idx[b]]
        goff = np.zeros((1, 2 * KEEP), np.int32)
        soff = np.zeros((1, 2 * KEEP), np.int32)
        color = ((sy // 8) % 2) * 2 + (sx // 8) % 2
        for c in range(2):
            goff[0, c::2] = ((sy + 8 * c) * W + sx) * DIM
            soff[0, c::2] = (color * NTOK + (sy + 8 * c) * W + sx) * DIM
        in_maps.append(
            {
                "x8": np.ascontiguousarray(x[b] * S_P).astype(f8np),
                "wqkv8": wqkv8,
                "woutb": woutb,
                "tkT8": tkT8,
                "gsidx": gs,
                "goff": goff,
                "soff": soff,
                "invp": (iv / np.float32(S_O * S_W)).astype(np.float32),
            }
        )
    return in_maps


def kernel(x, prob, W_fix, W_qkv, W_out, b_out):
    x = np.asarray(x, dtype=np.float32)
    prob = np.asarray(prob, dtype=np.float32)
    W_fix = np.asarray(W_fix, dtype=np.float32)
    W_qkv = np.asarray(W_qkv, dtype=np.float32)
    W_out = np.asarray(W_out, dtype=np.float32)
    b_out = np.asarray(b_out, dtype=np.float32)

    idx = _nms_select_host(prob, W_fix)  # (NB, KEEP)

    nc = _get_program()
    in_maps = prepare_in_maps(
        {"x": x, "prob": prob, "W_fix": W_fix, "W_qkv": W_qkv, "W_out": W_out}
    )
    res = run_bass_kernel_spmd(nc, in_maps, core_ids=list(range(NB)))
    # device returns the window contributions in 4 color slabs (zero where
    # uncovered); their sum is the accumulated contribution; x added here
    out = x + np.stack(
        [
            sum(
                res.results[b][k].astype(np.float32)
                for k in res.results[b]
                if k.startswith("out") and k != "outw"
            )
            for b in range(NB)
        ],
        0,
    )

    if np.any(b_out != 0.0):
        # bias contributes b_out once per covered token (cnt*inv == 1 exactly)
        for b in range(NB):
            gs, _ = _host_aux(idx[b])
            mask = np.zeros(NTOK, np.float32)
            mask[gs.ravel()] = 1.0
            out[b] += mask[:, None] * b_out[None, :]
    return out
